# revision 8
# baseline (speedup 1.0000x reference)
"""Trainium2 Bass kernel: masked contrastive loss, SPMD over 8 NeuronCores.

Math (reference: CustomContrastiveLoss):
  q = l2norm(logits.reshape(N,D)); k = l2norm(labels.reshape(N,D))
  sim = q @ k.T / TAU;  valid = pad_mask;  pos = (ad_i == ad_j) & valid_i & valid_j
  loss = mean_{valid rows} [ lse_valid(sim_row) - lse_pos(sim_row) ]
  (has_pos == valid because the diagonal is always a positive for valid rows)

Strategy (v2):
  * Host sorts the valid samples by ad value (pure index manipulation).
    Invalid rows/cols drop out entirely and each row's positives become one
    contiguous column window after a per-core column rotation.
  * |sim| <= 1/TAU so exp(sim - 1/TAU) needs no per-row max -> single pass:
    loss_row = ln(S_all) - ln(S_pos) with S = sum exp(sim - 1/TAU).
  * All tiny metadata (row ads / row masks / pre-broadcast window ads) is
    packed into ONE host array -> one DMA dispatch instead of seven.
  * Label normalization: one fused square+row-sum (scalar_tensor_tensor with
    accum_out) per tile, split across DVE and GpSimd, batched Ln/Exp rsqrt on
    ScalarE, scale+bf16-cast split across DVE/GpSimd, PE identity-transpose,
    PSUM->SBUF copy split across DVE/GpSimd.
  * Main loop is emitted column-group-outer and interleaved with the label
    pipeline so PE matmuls + ScalarE exp(row-sum accum) start as soon as the
    first 1536 label columns are ready instead of after the whole prologue.
  * S_pos band = one GpSimd fused multiply+row-sum per row tile.
"""

import math
import os
import sys

for _p in ("/opt/trn_rl_repo", "/root/.axon_site/_ro/trn_rl_repo"):
    if os.path.isdir(_p) and _p not in sys.path:
        sys.path.append(_p)

import numpy as np

import concourse.bass as bass
import concourse.mybir as mybir
import concourse.tile as tile
from concourse.masks import make_identity
from concourse.bass_utils import run_bass_kernel_spmd

TAU = 0.05
INV_TAU = 1.0 / TAU
P = 128
D = 256
KC = D // P
NCORES = 8
CHUNK = 512
GR = 1536
F32 = mybir.dt.float32
BF16 = mybir.dt.bfloat16
AF = mybir.ActivationFunctionType
OP = mybir.AluOpType

# ---------------------------------------------------------------------------
# This walrus build rejects more than one sync-wait per instruction.  After
# Tile scheduling, hoist excess waits onto same-engine NOPs inserted right
# before the over-subscribed instruction (engine streams are sequential, so
# the waits still happen-before the instruction).
_MAXW = 1
_wsplit_n = [0]


def _split_excess_waits(nc):
    for f in nc.m.functions:
        for bb in f.blocks:
            insts = bb.instructions
            i = 0
            while i < len(insts):
                inst = insts[i]
                si = getattr(inst, "sync_info", None)
                if si is not None and si.on_wait and len(si.on_wait) > _MAXW:
                    waits = list(si.on_wait)
                    si.on_wait = waits[:_MAXW]
                    rest = waits[_MAXW:]
                    for j in range(0, len(rest), _MAXW):
                        _wsplit_n[0] += 1
                        nop = mybir.InstNoOp(
                            name=f"wsplit-{_wsplit_n[0]}", ins=[], outs=[]
                        )
                        nop.engine = inst.engine
                        nop.sync_info = mybir.SyncInfo(
                            on_wait=rest[j : j + _MAXW], on_update=[]
                        )
                        insts.insert(i, nop)
                        i += 1
                i += 1


def build_program(V, Vy, R, Wtot, win_starts):
    T = R // P
    nyt = Vy // P
    MW = 3 * T
    groups = []
    c0 = 0
    while c0 < V:
        w = min(GR, V - c0)
        groups.append((c0, w))
        c0 += w
    ngr = len(groups)
    # y DMA dispatch groups: small first for fast pipeline start
    ygroups = []
    j0 = 0
    for want in (2, 2):
        if j0 < nyt:
            c = min(want, nyt - j0)
            ygroups.append((j0, c))
            j0 += c
    while j0 < nyt:
        c = min(4, nyt - j0)
        if nyt - (j0 + c) == 1:  # avoid a trailing 1-tile group
            c += 1
        ygroups.append((j0, c))
        j0 += c
    max_yg = max(c for _, c in ygroups)

    nc = bass.Bass("TRN2", target_bir_lowering=False, debug=False)
    xs = nc.dram_tensor("xs", [R, D], F32, kind="ExternalInput")
    ys = nc.dram_tensor("ys", [Vy, D], F32, kind="ExternalInput")
    meta = nc.dram_tensor("meta", [P, MW + T * Wtot], F32, kind="ExternalInput")
    outp = nc.dram_tensor("partial", [P, T], F32, kind="ExternalOutput")

    with tile.TileContext(nc) as tc:
        with (
            tc.tile_pool(name="singles", bufs=1) as singles,
            tc.tile_pool(name="stage", bufs=3) as stage_pool,
            tc.tile_pool(name="xstage", bufs=1) as xstage_pool,
            tc.tile_pool(name="scaled", bufs=6) as scaled,
            tc.tile_pool(name="scr", bufs=6) as scr_pool,
            tc.tile_pool(name="tiny", bufs=8) as tiny,
            tc.tile_pool(name="band", bufs=2) as band,
            tc.tile_pool(name="ptr", bufs=2, space="PSUM") as ptr,
            tc.tile_pool(name="pmm", bufs=2, space="PSUM") as pmm,
        ):
            ident = singles.tile([P, P], BF16)
            make_identity(nc, ident[:])
            b_eps = singles.tile([P, 1], F32)
            nc.vector.memset(b_eps[:], 1e-24)
            b_shift = singles.tile([P, 1], F32)
            nc.vector.memset(b_shift[:], -INV_TAU)
            b_ln20 = singles.tile([P, 1], F32)
            nc.vector.memset(b_ln20[:], float(np.log(INV_TAU)))

            ysT = singles.tile([P, KC, Vy], BF16)
            qT = singles.tile([P, KC, R], BF16)
            kssq = singles.tile([P, nyt], F32)
            krs = singles.tile([P, nyt], F32)
            qssq = singles.tile([P, T], F32)
            qs20 = singles.tile([P, T], F32)
            sparts = singles.tile([P, T, ngr], F32)
            sall = singles.tile([P, T], F32)
            spos = singles.tile([P, T], F32)
            meta_s = singles.tile([P, MW + T * Wtot], F32)
            masks = singles.tile([P, T, Wtot], BF16)

            # ---- DMA dispatches, metadata first (1 dispatch), then x, then y
            nc.sync.dma_start(out=meta_s[:], in_=meta.ap())
            xst = xstage_pool.tile([P, T, D], F32)
            nc.scalar.dma_start(
                out=xst[:], in_=xs.ap().rearrange("(b p) d -> p b d", p=P)
            )
            ystages = []
            disp = (nc.sync, nc.scalar)
            for gi, (j0, cnt) in enumerate(ygroups):
                st = stage_pool.tile([P, max_yg, D], F32)
                src = ys.ap()[j0 * P : (j0 + cnt) * P, :]
                disp[gi % 2].dma_start(
                    out=st[:, :cnt, :], in_=src.rearrange("(b p) d -> p b d", p=P)
                )
                for b in range(cnt):
                    ystages.append(st[:, b, :])

            # ---- masks from pre-broadcast window ads (gpsimd, frees DVE)
            for t in range(T):
                nc.gpsimd.tensor_scalar(
                    out=masks[:, t, :],
                    in0=meta_s[:, MW + t * Wtot : MW + (t + 1) * Wtot],
                    scalar1=meta_s[:, t : t + 1],
                    scalar2=None,
                    op0=OP.is_equal,
                )

            def sumsq(eng, src, dst_col):
                junk = scr_pool.tile([P, D], F32)
                eng.scalar_tensor_tensor(
                    out=junk[:], in0=src, scalar=1.0, in1=src,
                    op0=OP.bypass, op1=OP.mult, accum_out=dst_col,
                )

            def rsqrt_batch(src, dst, lo, hi, bias):
                # dst[:, lo:hi] = exp(-0.5*ln(src + eps) + bias)
                lns = tiny.tile([P, hi - lo], F32)
                nc.scalar.activation(out=lns[:], in_=src[:, lo:hi], func=AF.Ln,
                                     bias=b_eps[:], scale=1.0)
                nc.scalar.activation(out=dst[:, lo:hi], in_=lns[:], func=AF.Exp,
                                     bias=bias, scale=-0.5)

            # ---- x pipeline (small, runs first so qs20 is ready early)
            for t in range(T):
                sumsq(nc.vector, xst[:, t, :], qssq[:, t : t + 1])
            rsqrt_batch(qssq, qs20, 0, T, b_ln20[:])
            for t in range(T):
                sc = scaled.tile([P, D], BF16)
                nc.gpsimd.tensor_copy(out=sc[:], in_=xst[:, t, :])
                pt = ptr.tile([P, KC, P], BF16)
                for kc in range(KC):
                    nc.tensor.transpose(pt[:, kc, :], sc[:, kc * P : (kc + 1) * P],
                                        ident[:])
                nc.scalar.copy(qT[:, :, t * P : (t + 1) * P], pt[:])

            est_tiles = [singles.tile([P, V], BF16, name=f"est{t}")
                         for t in range(T)]

            def emit_main_group(gi):
                c0, w = groups[gi]
                for t in range(T):
                    ps = pmm.tile([P, GR], F32)
                    for h in range(0, w, CHUNK):
                        hw = min(CHUNK, w - h)
                        for kc in range(KC):
                            nc.tensor.matmul(
                                ps[:, h : h + hw],
                                qT[:, kc, t * P : (t + 1) * P],
                                ysT[:, kc, c0 + h : c0 + h + hw],
                                start=(kc == 0), stop=(kc == KC - 1),
                            )
                    nc.scalar.activation(
                        out=est_tiles[t][:, c0 : c0 + w], in_=ps[:, :w],
                        func=AF.Exp, bias=b_shift[:],
                        scale=qs20[:, t : t + 1],
                        accum_out=sparts[:, t, gi : gi + 1],
                    )

            need_tiles = [(c0 + w + P - 1) // P for (c0, w) in groups]

            # ---- y pipeline interleaved with main-loop emission
            NB = 8
            main_emitted = 0
            for j in range(nyt):
                sumsq(nc.vector, ystages[j], kssq[:, j : j + 1])
                if j % NB == NB - 1 or j == nyt - 1:
                    lo = (j // NB) * NB
                    rsqrt_batch(kssq, krs, lo, j + 1, 0.0)
                    # transpose in bursts of up to 4 tiles into one PSUM bank,
                    # then drain with a single wide DVE copy (gpsimd has no
                    # PSUM port, so copies stay on DVE/ScalarE)
                    for b0 in range(lo, j + 1, 4):
                        b1 = min(b0 + 4, j + 1)
                        pt = ptr.tile([P, KC, 4 * P], BF16)
                        for jj in range(b0, b1):
                            sc = scaled.tile([P, D], BF16)
                            nc.gpsimd.tensor_scalar_mul(sc[:], ystages[jj],
                                                        krs[:, jj : jj + 1])
                            o = (jj - b0) * P
                            for kc in range(KC):
                                nc.tensor.transpose(
                                    pt[:, kc, o : o + P],
                                    sc[:, kc * P : (kc + 1) * P], ident[:]
                                )
                        nc.vector.tensor_copy(
                            out=ysT[:, :, b0 * P : b1 * P],
                            in_=pt[:, :, : (b1 - b0) * P])
                    while (main_emitted < ngr
                           and need_tiles[main_emitted] <= j + 1):
                        emit_main_group(main_emitted)
                        main_emitted += 1
            while main_emitted < ngr:
                emit_main_group(main_emitted)
                main_emitted += 1

            # ---- bands (gpsimd fused mult+sum) + S_all reduce (DVE)
            for t in range(T):
                w0 = win_starts[t]
                bscr = band.tile([P, Wtot], F32)
                nc.vector.scalar_tensor_tensor(
                    out=bscr[:], in0=est_tiles[t][:, w0 : w0 + Wtot],
                    scalar=1.0, in1=masks[:, t, :],
                    op0=OP.bypass, op1=OP.mult,
                    accum_out=spos[:, t : t + 1],
                )
                nc.vector.tensor_reduce(out=sall[:, t : t + 1],
                                        in_=sparts[:, t, :],
                                        axis=mybir.AxisListType.X, op=OP.add)

            # ---- batched epilogue: loss rows, mask, partition sum
            sposg = tiny.tile([P, T], F32)
            nc.vector.tensor_add(out=sposg[:], in0=spos[:],
                                 in1=meta_s[:, 2 * T : 3 * T])
            lall = tiny.tile([P, T], F32)
            nc.scalar.activation(out=lall[:], in_=sall[:], func=AF.Ln,
                                 bias=0.0, scale=1.0)
            lpos = tiny.tile([P, T], F32)
            nc.scalar.activation(out=lpos[:], in_=sposg[:], func=AF.Ln,
                                 bias=0.0, scale=1.0)
            dls = tiny.tile([P, T], F32)
            nc.vector.tensor_sub(out=dls[:], in0=lall[:], in1=lpos[:])
            dlm = tiny.tile([P, T], F32)
            nc.vector.tensor_mul(out=dlm[:], in0=dls[:], in1=meta_s[:, T : 2 * T])
            nc.sync.dma_start(out=outp.ap(), in_=dlm[:])

    return nc


def _roundup(a, b):
    return (a + b - 1) // b * b


def plan(valid, ad):
    """Host-side sharding plan from the pad mask / ad ids (index math only)."""
    idx = np.nonzero(valid)[0]
    V = int(idx.size)
    if V == 0:
        return None
    order = idx[np.argsort(ad[idx], kind="stable")]
    ads = ad[order].astype(np.int64)
    R = _roundup(_roundup(V, NCORES) // NCORES, P)
    Vy = _roundup(V, P)
    W = int(np.bincount(ads).max())
    Wtot = min(_roundup(2 * W + P, 32), V)
    T = R // P
    rotate = (R - P + Wtot <= V) and Wtot < V
    if rotate:
        win_starts = tuple(min(t * P, V - Wtot) for t in range(T))
    else:
        Wtot = V
        win_starts = (0,) * T
    return dict(V=V, Vy=Vy, R=R, T=T, W=W, Wtot=Wtot, win_starts=win_starts,
                rotate=rotate, order=order, ads=ads)


def core_inputs(pl, x, y, c):
    """Build core c's input arrays from the plan (host indexing only)."""
    V, R, Vy, W, T, Wtot = (pl["V"], pl["R"], pl["Vy"], pl["W"], pl["T"],
                            pl["Wtot"])
    order, ads, win_starts = pl["order"], pl["ads"], pl["win_starts"]
    g0 = c * R
    take = order[g0 : g0 + R]
    xs = np.zeros((R, D), np.float32)
    xs[: take.size] = x[take]
    adr_flat = np.full(R, -1.0, np.float32)
    adr_flat[: take.size] = ads[g0 : g0 + take.size]
    n_valid = max(0, min(R, V - g0))
    rmask_flat = np.zeros(R, np.float32)
    rmask_flat[:n_valid] = 1.0
    # packed [P, T]: column t holds rows [t*P, (t+1)*P) of this core's shard
    adr = np.ascontiguousarray(adr_flat.reshape(T, P).T)
    rmask = np.ascontiguousarray(rmask_flat.reshape(T, P).T)
    rpad = np.ascontiguousarray(1.0 - rmask)
    if pl["rotate"]:
        colsel = (np.arange(V) + g0 - W) % V
    else:
        colsel = np.arange(V)
    cols = order[colsel]
    ys = np.zeros((Vy, D), np.float32)
    ys[:V] = y[cols]
    adc = ads[colsel].astype(np.float32)
    MW = 3 * T
    meta = np.zeros((P, MW + T * Wtot), np.float32)
    meta[:, 0:T] = adr
    meta[:, T : 2 * T] = rmask
    meta[:, 2 * T : 3 * T] = rpad
    for t in range(T):
        w0 = win_starts[t]
        meta[:, MW + t * Wtot : MW + (t + 1) * Wtot] = adc[w0 : w0 + Wtot][None, :]
    return {"xs": xs, "ys": ys, "meta": meta}


_prog_cache = {}


def _get_program(key_pl):
    key = (key_pl["V"], key_pl["Vy"], key_pl["R"],
           key_pl["Wtot"], key_pl["win_starts"])
    if key not in _prog_cache:
        _prog_cache[key] = build_program(
            key_pl["V"], key_pl["Vy"], key_pl["R"],
            key_pl["Wtot"], key_pl["win_starts"]
        )
    return _prog_cache[key]


def kernel(logits, labels, pad_mask, ad_idxs, _want_results=False, **run_kwargs):
    x = np.ascontiguousarray(np.asarray(logits), dtype=np.float32).reshape(-1, D)
    y = np.ascontiguousarray(np.asarray(labels), dtype=np.float32).reshape(-1, D)
    valid = np.asarray(pad_mask).reshape(-1).astype(bool)
    ad = np.asarray(ad_idxs).reshape(-1).astype(np.int64)

    pl = plan(valid, ad)
    if pl is None:
        return np.float32(0.0)

    nc = _get_program(pl)
    # CoreSim chokes on the inserted NOPs, so split waits only for the HW path
    if not getattr(nc, "_waits_split", False):
        _split_excess_waits(nc)
        nc._waits_split = True
    in_maps = [core_inputs(pl, x, y, c) for c in range(NCORES)]
    res = run_bass_kernel_spmd(nc, in_maps, core_ids=list(range(NCORES)),
                               **run_kwargs)
    total = sum(float(res.results[c]["partial"].sum()) for c in range(NCORES))
    loss = np.float32(total / pl["V"])
    if _want_results:
        return loss, res
    return loss


# revision 9
# speedup vs baseline: 2.5876x; 2.5876x over previous
"""Trainium2 Bass kernel: masked contrastive loss, SPMD over 8 NeuronCores.

Math (reference: CustomContrastiveLoss):
  q = l2norm(logits.reshape(N,D)); k = l2norm(labels.reshape(N,D))
  sim = q @ k.T / TAU;  valid = pad_mask;  pos = (ad_i == ad_j) & valid_i & valid_j
  loss = mean_{valid rows} [ lse_valid(sim_row) - lse_pos(sim_row) ]
  (has_pos == valid because the diagonal is always a positive for valid rows)

Strategy (v2):
  * Host sorts the valid samples by ad value (pure index manipulation).
    Invalid rows/cols drop out entirely and each row's positives become one
    contiguous column window after a per-core column rotation.
  * |sim| <= 1/TAU so exp(sim - 1/TAU) needs no per-row max -> single pass:
    loss_row = ln(S_all) - ln(S_pos) with S = sum exp(sim - 1/TAU).
  * All tiny metadata (row ads / row masks / pre-broadcast window ads) is
    packed into ONE host array -> one DMA dispatch instead of seven.
  * Label normalization: one fused square+row-sum (scalar_tensor_tensor with
    accum_out) per tile, split across DVE and GpSimd, batched Ln/Exp rsqrt on
    ScalarE, scale+bf16-cast split across DVE/GpSimd, PE identity-transpose,
    PSUM->SBUF copy split across DVE/GpSimd.
  * Main loop is emitted column-group-outer and interleaved with the label
    pipeline so PE matmuls + ScalarE exp(row-sum accum) start as soon as the
    first 1536 label columns are ready instead of after the whole prologue.
  * S_pos band = one GpSimd fused multiply+row-sum per row tile.
"""

import math
import os
import sys

for _p in ("/opt/trn_rl_repo", "/root/.axon_site/_ro/trn_rl_repo"):
    if os.path.isdir(_p) and _p not in sys.path:
        sys.path.append(_p)

import numpy as np

import concourse.bass as bass
import concourse.mybir as mybir
import concourse.tile as tile
from concourse.masks import make_identity
from concourse.bass_utils import run_bass_kernel_spmd

TAU = 0.05
INV_TAU = 1.0 / TAU
P = 128
D = 256
KC = D // P
NCORES = 8
CHUNK = 512
GR = 1536
F32 = mybir.dt.float32
BF16 = mybir.dt.bfloat16
AF = mybir.ActivationFunctionType
OP = mybir.AluOpType

# ---------------------------------------------------------------------------
# This walrus build rejects more than one sync-wait per instruction.  After
# Tile scheduling, hoist excess waits onto same-engine NOPs inserted right
# before the over-subscribed instruction (engine streams are sequential, so
# the waits still happen-before the instruction).
_MAXW = 1
_wsplit_n = [0]


def _split_excess_waits(nc):
    for f in nc.m.functions:
        for bb in f.blocks:
            insts = bb.instructions
            i = 0
            while i < len(insts):
                inst = insts[i]
                si = getattr(inst, "sync_info", None)
                if si is not None and si.on_wait and len(si.on_wait) > _MAXW:
                    waits = list(si.on_wait)
                    si.on_wait = waits[:_MAXW]
                    rest = waits[_MAXW:]
                    for j in range(0, len(rest), _MAXW):
                        _wsplit_n[0] += 1
                        nop = mybir.InstNoOp(
                            name=f"wsplit-{_wsplit_n[0]}", ins=[], outs=[]
                        )
                        nop.engine = inst.engine
                        nop.sync_info = mybir.SyncInfo(
                            on_wait=rest[j : j + _MAXW], on_update=[]
                        )
                        insts.insert(i, nop)
                        i += 1
                i += 1


def build_program(V, Vy, R, Wtot, win_starts):
    T = R // P
    nyt = Vy // P
    MW = 3 * T
    groups = []
    c0 = 0
    for want in (512, 1024):
        if c0 < V:
            w = min(want, V - c0)
            groups.append((c0, w))
            c0 += w
    while c0 < V:
        w = min(GR, V - c0)
        groups.append((c0, w))
        c0 += w
    ngr = len(groups)
    # y DMA dispatch groups: small first for fast pipeline start
    ygroups = []
    j0 = 0
    for want in (2, 2):
        if j0 < nyt:
            c = min(want, nyt - j0)
            ygroups.append((j0, c))
            j0 += c
    while j0 < nyt:
        c = min(4, nyt - j0)
        if nyt - (j0 + c) == 1:  # avoid a trailing 1-tile group
            c += 1
        ygroups.append((j0, c))
        j0 += c
    max_yg = max(c for _, c in ygroups)

    nc = bass.Bass("TRN2", target_bir_lowering=False, debug=False)
    xs = nc.dram_tensor("xs", [R, D], F32, kind="ExternalInput")
    ys = nc.dram_tensor("ys", [Vy, D], F32, kind="ExternalInput")
    meta = nc.dram_tensor("meta", [P, MW + T * Wtot], F32, kind="ExternalInput")
    outp = nc.dram_tensor("partial", [P, T], F32, kind="ExternalOutput")

    with tile.TileContext(nc) as tc:
        with (
            tc.tile_pool(name="singles", bufs=1) as singles,
            tc.tile_pool(name="stage", bufs=3) as stage_pool,
            tc.tile_pool(name="xstage", bufs=1) as xstage_pool,
            tc.tile_pool(name="scaled", bufs=6) as scaled,
            tc.tile_pool(name="scr", bufs=6) as scr_pool,
            tc.tile_pool(name="tiny", bufs=8) as tiny,
            tc.tile_pool(name="band", bufs=2) as band,
            tc.tile_pool(name="ptr", bufs=2, space="PSUM") as ptr,
            tc.tile_pool(name="pmm", bufs=2, space="PSUM") as pmm,
        ):
            ident = singles.tile([P, P], BF16)
            make_identity(nc, ident[:])
            b_eps = singles.tile([P, 1], F32)
            nc.vector.memset(b_eps[:], 1e-24)
            b_shift = singles.tile([P, 1], F32)
            nc.vector.memset(b_shift[:], -INV_TAU)
            b_ln20 = singles.tile([P, 1], F32)
            nc.vector.memset(b_ln20[:], float(np.log(INV_TAU)))

            ysT = singles.tile([P, KC, Vy], BF16)
            qT = singles.tile([P, KC, R], BF16)
            kssq = singles.tile([P, nyt], F32)
            krs = singles.tile([P, nyt], F32)
            qssq = singles.tile([P, T], F32)
            qs20 = singles.tile([P, T], F32)
            sparts = singles.tile([P, T, ngr], F32)
            sall = singles.tile([P, T], F32)
            spos = singles.tile([P, T], F32)
            meta_s = singles.tile([P, MW + T * Wtot], F32)
            masks = singles.tile([P, T, Wtot], BF16)

            # ---- DMA dispatches, metadata first (1 dispatch), then x, then y
            nc.sync.dma_start(out=meta_s[:], in_=meta.ap())
            xst = xstage_pool.tile([P, T, D], F32)
            nc.scalar.dma_start(
                out=xst[:], in_=xs.ap().rearrange("(b p) d -> p b d", p=P)
            )
            ystages = []
            disp = (nc.sync, nc.scalar)
            for gi, (j0, cnt) in enumerate(ygroups):
                st = stage_pool.tile([P, max_yg, D], F32)
                src = ys.ap()[j0 * P : (j0 + cnt) * P, :]
                disp[gi % 2].dma_start(
                    out=st[:, :cnt, :], in_=src.rearrange("(b p) d -> p b d", p=P)
                )
                for b in range(cnt):
                    ystages.append(st[:, b, :])

            # ---- masks from pre-broadcast window ads (gpsimd, frees DVE)
            for t in range(T):
                nc.gpsimd.tensor_scalar(
                    out=masks[:, t, :],
                    in0=meta_s[:, MW + t * Wtot : MW + (t + 1) * Wtot],
                    scalar1=meta_s[:, t : t + 1],
                    scalar2=None,
                    op0=OP.is_equal,
                )

            def sumsq(eng, src, dst_col):
                junk = scr_pool.tile([P, D], F32)
                eng.scalar_tensor_tensor(
                    out=junk[:], in0=src, scalar=1.0, in1=src,
                    op0=OP.bypass, op1=OP.mult, accum_out=dst_col,
                )

            def rsqrt_batch(src, dst, lo, hi, bias):
                # dst[:, lo:hi] = exp(-0.5*ln(src + eps) + bias)
                lns = tiny.tile([P, hi - lo], F32)
                nc.scalar.activation(out=lns[:], in_=src[:, lo:hi], func=AF.Ln,
                                     bias=b_eps[:], scale=1.0)
                nc.scalar.activation(out=dst[:, lo:hi], in_=lns[:], func=AF.Exp,
                                     bias=bias, scale=-0.5)

            # ---- x pipeline on ScalarE (idle before exp; frees DVE)
            for t in range(T):
                junk = scr_pool.tile([P, D], F32)
                nc.scalar.activation(out=junk[:], in_=xst[:, t, :],
                                     func=AF.Square,
                                     accum_out=qssq[:, t : t + 1])
            rsqrt_batch(qssq, qs20, 0, T, b_ln20[:])
            for t in range(T):
                sc = scaled.tile([P, D], BF16)
                nc.scalar.copy(sc[:], xst[:, t, :])
                pt = ptr.tile([P, KC, P], BF16)
                for kc in range(KC):
                    nc.tensor.transpose(pt[:, kc, :], sc[:, kc * P : (kc + 1) * P],
                                        ident[:])
                nc.scalar.copy(qT[:, :, t * P : (t + 1) * P], pt[:])

            est_tiles = [singles.tile([P, V], BF16, name=f"est{t}")
                         for t in range(T)]

            def emit_main_group(gi):
                c0, w = groups[gi]
                for t in range(T):
                    ps = pmm.tile([P, GR], F32)
                    for h in range(0, w, CHUNK):
                        hw = min(CHUNK, w - h)
                        for kc in range(KC):
                            nc.tensor.matmul(
                                ps[:, h : h + hw],
                                qT[:, kc, t * P : (t + 1) * P],
                                ysT[:, kc, c0 + h : c0 + h + hw],
                                start=(kc == 0), stop=(kc == KC - 1),
                            )
                    nc.scalar.activation(
                        out=est_tiles[t][:, c0 : c0 + w], in_=ps[:, :w],
                        func=AF.Exp, bias=b_shift[:],
                        scale=qs20[:, t : t + 1],
                        accum_out=sparts[:, t, gi : gi + 1],
                    )

            need_tiles = [(c0 + w + P - 1) // P for (c0, w) in groups]

            # ---- y pipeline interleaved with main-loop emission
            NB = 8
            main_emitted = 0
            for j in range(nyt):
                sumsq(nc.vector, ystages[j], kssq[:, j : j + 1])
                if j % NB == NB - 1 or j == nyt - 1:
                    lo = (j // NB) * NB
                    rsqrt_batch(kssq, krs, lo, j + 1, 0.0)
                    # transpose in bursts of up to 4 tiles into one PSUM bank,
                    # then drain with a single wide DVE copy (gpsimd has no
                    # PSUM port, so copies stay on DVE/ScalarE)
                    for b0 in range(lo, j + 1, 4):
                        b1 = min(b0 + 4, j + 1)
                        pt = ptr.tile([P, KC, 4 * P], BF16)
                        for jj in range(b0, b1):
                            sc = scaled.tile([P, D], BF16)
                            nc.vector.tensor_scalar_mul(sc[:], ystages[jj],
                                                         krs[:, jj : jj + 1])
                            o = (jj - b0) * P
                            for kc in range(KC):
                                nc.tensor.transpose(
                                    pt[:, kc, o : o + P],
                                    sc[:, kc * P : (kc + 1) * P], ident[:]
                                )
                        nc.vector.tensor_copy(
                            out=ysT[:, :, b0 * P : b1 * P],
                            in_=pt[:, :, : (b1 - b0) * P])
                    while (main_emitted < ngr
                           and need_tiles[main_emitted] <= j + 1):
                        emit_main_group(main_emitted)
                        main_emitted += 1
            while main_emitted < ngr:
                emit_main_group(main_emitted)
                main_emitted += 1

            # ---- bands (gpsimd fused mult+sum) + S_all reduce (DVE)
            for t in range(T):
                w0 = win_starts[t]
                bscr = band.tile([P, Wtot], F32)
                nc.vector.scalar_tensor_tensor(
                    out=bscr[:], in0=est_tiles[t][:, w0 : w0 + Wtot],
                    scalar=1.0, in1=masks[:, t, :],
                    op0=OP.bypass, op1=OP.mult,
                    accum_out=spos[:, t : t + 1],
                )
                nc.vector.tensor_reduce(out=sall[:, t : t + 1],
                                        in_=sparts[:, t, :],
                                        axis=mybir.AxisListType.X, op=OP.add)

            # ---- batched epilogue: loss rows, mask, partition sum
            sposg = tiny.tile([P, T], F32)
            nc.vector.tensor_add(out=sposg[:], in0=spos[:],
                                 in1=meta_s[:, 2 * T : 3 * T])
            lall = tiny.tile([P, T], F32)
            nc.scalar.activation(out=lall[:], in_=sall[:], func=AF.Ln,
                                 bias=0.0, scale=1.0)
            lpos = tiny.tile([P, T], F32)
            nc.scalar.activation(out=lpos[:], in_=sposg[:], func=AF.Ln,
                                 bias=0.0, scale=1.0)
            dls = tiny.tile([P, T], F32)
            nc.vector.tensor_sub(out=dls[:], in0=lall[:], in1=lpos[:])
            dlm = tiny.tile([P, T], F32)
            nc.vector.tensor_mul(out=dlm[:], in0=dls[:], in1=meta_s[:, T : 2 * T])
            nc.sync.dma_start(out=outp.ap(), in_=dlm[:])

    return nc


def _roundup(a, b):
    return (a + b - 1) // b * b


def plan(valid, ad):
    """Host-side sharding plan from the pad mask / ad ids (index math only)."""
    idx = np.nonzero(valid)[0]
    V = int(idx.size)
    if V == 0:
        return None
    order = idx[np.argsort(ad[idx], kind="stable")]
    ads = ad[order].astype(np.int64)
    R = _roundup(_roundup(V, NCORES) // NCORES, P)
    Vy = _roundup(V, P)
    W = int(np.bincount(ads).max())
    Wtot = min(_roundup(2 * W + P, 32), V)
    T = R // P
    rotate = (R - P + Wtot <= V) and Wtot < V
    if rotate:
        win_starts = tuple(min(t * P, V - Wtot) for t in range(T))
    else:
        Wtot = V
        win_starts = (0,) * T
    return dict(V=V, Vy=Vy, R=R, T=T, W=W, Wtot=Wtot, win_starts=win_starts,
                rotate=rotate, order=order, ads=ads)


def core_inputs(pl, x, y, c):
    """Build core c's input arrays from the plan (host indexing only)."""
    V, R, Vy, W, T, Wtot = (pl["V"], pl["R"], pl["Vy"], pl["W"], pl["T"],
                            pl["Wtot"])
    order, ads, win_starts = pl["order"], pl["ads"], pl["win_starts"]
    g0 = c * R
    take = order[g0 : g0 + R]
    xs = np.zeros((R, D), np.float32)
    xs[: take.size] = x[take]
    adr_flat = np.full(R, -1.0, np.float32)
    adr_flat[: take.size] = ads[g0 : g0 + take.size]
    n_valid = max(0, min(R, V - g0))
    rmask_flat = np.zeros(R, np.float32)
    rmask_flat[:n_valid] = 1.0
    # packed [P, T]: column t holds rows [t*P, (t+1)*P) of this core's shard
    adr = np.ascontiguousarray(adr_flat.reshape(T, P).T)
    rmask = np.ascontiguousarray(rmask_flat.reshape(T, P).T)
    rpad = np.ascontiguousarray(1.0 - rmask)
    if pl["rotate"]:
        colsel = (np.arange(V) + g0 - W) % V
    else:
        colsel = np.arange(V)
    cols = order[colsel]
    ys = np.zeros((Vy, D), np.float32)
    ys[:V] = y[cols]
    adc = ads[colsel].astype(np.float32)
    MW = 3 * T
    meta = np.zeros((P, MW + T * Wtot), np.float32)
    meta[:, 0:T] = adr
    meta[:, T : 2 * T] = rmask
    meta[:, 2 * T : 3 * T] = rpad
    for t in range(T):
        w0 = win_starts[t]
        meta[:, MW + t * Wtot : MW + (t + 1) * Wtot] = adc[w0 : w0 + Wtot][None, :]
    return {"xs": xs, "ys": ys, "meta": meta}


_prog_cache = {}


def _get_program(key_pl):
    key = (key_pl["V"], key_pl["Vy"], key_pl["R"],
           key_pl["Wtot"], key_pl["win_starts"])
    if key not in _prog_cache:
        _prog_cache[key] = build_program(
            key_pl["V"], key_pl["Vy"], key_pl["R"],
            key_pl["Wtot"], key_pl["win_starts"]
        )
    return _prog_cache[key]


def kernel(logits, labels, pad_mask, ad_idxs, _want_results=False, **run_kwargs):
    x = np.ascontiguousarray(np.asarray(logits), dtype=np.float32).reshape(-1, D)
    y = np.ascontiguousarray(np.asarray(labels), dtype=np.float32).reshape(-1, D)
    valid = np.asarray(pad_mask).reshape(-1).astype(bool)
    ad = np.asarray(ad_idxs).reshape(-1).astype(np.int64)

    pl = plan(valid, ad)
    if pl is None:
        return np.float32(0.0)

    nc = _get_program(pl)
    # CoreSim chokes on the inserted NOPs, so split waits only for the HW path
    if not getattr(nc, "_waits_split", False):
        _split_excess_waits(nc)
        nc._waits_split = True
    in_maps = [core_inputs(pl, x, y, c) for c in range(NCORES)]
    res = run_bass_kernel_spmd(nc, in_maps, core_ids=list(range(NCORES)),
                               **run_kwargs)
    total = sum(float(res.results[c]["partial"].sum()) for c in range(NCORES))
    loss = np.float32(total / pl["V"])
    if _want_results:
        return loss, res
    return loss


# revision 10
# speedup vs baseline: 3.0069x; 1.1620x over previous
"""Trainium2 Bass kernel: masked contrastive loss, SPMD over 8 NeuronCores.

Math (reference: CustomContrastiveLoss):
  q = l2norm(logits.reshape(N,D)); k = l2norm(labels.reshape(N,D))
  sim = q @ k.T / TAU;  valid = pad_mask;  pos = (ad_i == ad_j) & valid_i & valid_j
  loss = mean_{valid rows} [ lse_valid(sim_row) - lse_pos(sim_row) ]
  (has_pos == valid because the diagonal is always a positive for valid rows)

Strategy (v2):
  * Host sorts the valid samples by ad value (pure index manipulation).
    Invalid rows/cols drop out entirely and each row's positives become one
    contiguous column window after a per-core column rotation.
  * |sim| <= 1/TAU so exp(sim - 1/TAU) needs no per-row max -> single pass:
    loss_row = ln(S_all) - ln(S_pos) with S = sum exp(sim - 1/TAU).
  * All tiny metadata (row ads / row masks / pre-broadcast window ads) is
    packed into ONE host array -> one DMA dispatch instead of seven.
  * Label normalization: one fused square+row-sum (scalar_tensor_tensor with
    accum_out) per tile, split across DVE and GpSimd, batched Ln/Exp rsqrt on
    ScalarE, scale+bf16-cast split across DVE/GpSimd, PE identity-transpose,
    PSUM->SBUF copy split across DVE/GpSimd.
  * Main loop is emitted column-group-outer and interleaved with the label
    pipeline so PE matmuls + ScalarE exp(row-sum accum) start as soon as the
    first 1536 label columns are ready instead of after the whole prologue.
  * S_pos band = one GpSimd fused multiply+row-sum per row tile.
"""

import math
import os
import sys

for _p in ("/opt/trn_rl_repo", "/root/.axon_site/_ro/trn_rl_repo"):
    if os.path.isdir(_p) and _p not in sys.path:
        sys.path.append(_p)

import numpy as np

import concourse.bass as bass
import concourse.mybir as mybir
import concourse.tile as tile
from concourse.masks import make_identity
from concourse.bass_utils import run_bass_kernel_spmd

TAU = 0.05
INV_TAU = 1.0 / TAU
P = 128
D = 256
KC = D // P
NCORES = 8
CHUNK = 512
GR = 1536
F32 = mybir.dt.float32
BF16 = mybir.dt.bfloat16
AF = mybir.ActivationFunctionType
OP = mybir.AluOpType

# ---------------------------------------------------------------------------
# This walrus build rejects more than one sync-wait per instruction.  After
# Tile scheduling, hoist excess waits onto same-engine NOPs inserted right
# before the over-subscribed instruction (engine streams are sequential, so
# the waits still happen-before the instruction).
_MAXW = 1
_wsplit_n = [0]


def _split_excess_waits(nc):
    for f in nc.m.functions:
        for bb in f.blocks:
            insts = bb.instructions
            i = 0
            while i < len(insts):
                inst = insts[i]
                si = getattr(inst, "sync_info", None)
                if si is not None and si.on_wait and len(si.on_wait) > _MAXW:
                    waits = list(si.on_wait)
                    si.on_wait = waits[:_MAXW]
                    rest = waits[_MAXW:]
                    for j in range(0, len(rest), _MAXW):
                        _wsplit_n[0] += 1
                        nop = mybir.InstNoOp(
                            name=f"wsplit-{_wsplit_n[0]}", ins=[], outs=[]
                        )
                        nop.engine = inst.engine
                        nop.sync_info = mybir.SyncInfo(
                            on_wait=rest[j : j + _MAXW], on_update=[]
                        )
                        insts.insert(i, nop)
                        i += 1
                i += 1


def build_program(V, Vy, R, Wtot, win_starts):
    T = R // P
    nyt = Vy // P
    MW = 3 * T
    groups = []
    c0 = 0
    for want in (512, 1024):
        if c0 < V:
            w = min(want, V - c0)
            groups.append((c0, w))
            c0 += w
    while c0 < V:
        w = min(GR, V - c0)
        groups.append((c0, w))
        c0 += w
    ngr = len(groups)
    # y DMA dispatch groups: small first for fast pipeline start
    ygroups = []
    j0 = 0
    for want in (2, 2):
        if j0 < nyt:
            c = min(want, nyt - j0)
            ygroups.append((j0, c))
            j0 += c
    while j0 < nyt:
        c = min(4, nyt - j0)
        if nyt - (j0 + c) == 1:  # avoid a trailing 1-tile group
            c += 1
        ygroups.append((j0, c))
        j0 += c
    max_yg = max(c for _, c in ygroups)

    nc = bass.Bass("TRN2", target_bir_lowering=False, debug=False)
    xs = nc.dram_tensor("xs", [P, T * D], F32, kind="ExternalInput")
    ys = nc.dram_tensor("ys", [P, nyt * D], F32, kind="ExternalInput")
    meta = nc.dram_tensor("meta", [P, MW + T * Wtot], F32, kind="ExternalInput")
    outp = nc.dram_tensor("partial", [P, T], F32, kind="ExternalOutput")

    with tile.TileContext(nc) as tc:
        with (
            tc.tile_pool(name="singles", bufs=1) as singles,
            tc.tile_pool(name="stage", bufs=6) as stage_pool,
            tc.tile_pool(name="xstage", bufs=1) as xstage_pool,
            tc.tile_pool(name="scaled", bufs=6) as scaled,
            tc.tile_pool(name="scr", bufs=6) as scr_pool,
            tc.tile_pool(name="tiny", bufs=8) as tiny,
            tc.tile_pool(name="band", bufs=2) as band,
            tc.tile_pool(name="ptr", bufs=2, space="PSUM") as ptr,
            tc.tile_pool(name="pmm", bufs=2, space="PSUM") as pmm,
        ):
            ident = singles.tile([P, P], BF16)
            make_identity(nc, ident[:])
            b_eps = singles.tile([P, 1], F32)
            nc.vector.memset(b_eps[:], 1e-24)
            b_shift = singles.tile([P, 1], F32)
            nc.vector.memset(b_shift[:], -INV_TAU)
            b_ln20 = singles.tile([P, 1], F32)
            nc.vector.memset(b_ln20[:], float(np.log(INV_TAU)))

            ysT = singles.tile([P, KC, Vy], BF16)
            qT = singles.tile([P, KC, R], BF16)
            kssq = singles.tile([P, nyt], F32)
            krs = singles.tile([P, nyt], F32)
            qssq = singles.tile([P, T], F32)
            qs20 = singles.tile([P, T], F32)
            sparts = singles.tile([P, T, ngr], F32)
            sall = singles.tile([P, T], F32)
            spos = singles.tile([P, T], F32)
            meta_s = singles.tile([P, MW + T * Wtot], F32)
            masks = singles.tile([P, T, Wtot], BF16)

            # ---- DMA dispatches, metadata first (1 dispatch), then x, then y
            nc.sync.dma_start(out=meta_s[:], in_=meta.ap())
            xst = xstage_pool.tile([P, T, D], F32)
            nc.scalar.dma_start(
                out=xst[:], in_=xs.ap().rearrange("p (b d) -> p b d", d=D)
            )
            ystages = []
            disp = (nc.sync, nc.scalar)
            for gi, (j0, cnt) in enumerate(ygroups):
                st = stage_pool.tile([P, max_yg, D], F32)
                src = ys.ap()[:, j0 * D : (j0 + cnt) * D]
                disp[gi % 2].dma_start(
                    out=st[:, :cnt, :], in_=src.rearrange("p (b d) -> p b d", d=D)
                )
                for b in range(cnt):
                    ystages.append(st[:, b, :])

            # ---- masks from pre-broadcast window ads (gpsimd, frees DVE)
            for t in range(T):
                nc.gpsimd.tensor_scalar(
                    out=masks[:, t, :],
                    in0=meta_s[:, MW + t * Wtot : MW + (t + 1) * Wtot],
                    scalar1=meta_s[:, t : t + 1],
                    scalar2=None,
                    op0=OP.is_equal,
                )

            def sumsq(eng, src, dst_col):
                junk = scr_pool.tile([P, D], F32)
                eng.scalar_tensor_tensor(
                    out=junk[:], in0=src, scalar=1.0, in1=src,
                    op0=OP.bypass, op1=OP.mult, accum_out=dst_col,
                )

            def rsqrt_batch(src, dst, lo, hi, bias):
                # dst[:, lo:hi] = exp(-0.5*ln(src + eps) + bias)
                lns = tiny.tile([P, hi - lo], F32)
                nc.scalar.activation(out=lns[:], in_=src[:, lo:hi], func=AF.Ln,
                                     bias=b_eps[:], scale=1.0)
                nc.scalar.activation(out=dst[:, lo:hi], in_=lns[:], func=AF.Exp,
                                     bias=bias, scale=-0.5)

            # ---- x pipeline on ScalarE (idle before exp; frees DVE)
            for t in range(T):
                junk = scr_pool.tile([P, D], F32)
                nc.scalar.activation(out=junk[:], in_=xst[:, t, :],
                                     func=AF.Square,
                                     accum_out=qssq[:, t : t + 1])
            rsqrt_batch(qssq, qs20, 0, T, b_ln20[:])
            for t in range(T):
                sc = scaled.tile([P, D], BF16)
                nc.scalar.copy(sc[:], xst[:, t, :])
                pt = ptr.tile([P, KC, P], BF16)
                for kc in range(KC):
                    nc.tensor.transpose(pt[:, kc, :], sc[:, kc * P : (kc + 1) * P],
                                        ident[:])
                nc.scalar.copy(qT[:, :, t * P : (t + 1) * P], pt[:])

            est_tiles = [singles.tile([P, V], BF16, name=f"est{t}")
                         for t in range(T)]

            def emit_main_group(gi):
                c0, w = groups[gi]
                for t in range(T):
                    ps = pmm.tile([P, GR], F32)
                    for h in range(0, w, CHUNK):
                        hw = min(CHUNK, w - h)
                        for kc in range(KC):
                            nc.tensor.matmul(
                                ps[:, h : h + hw],
                                qT[:, kc, t * P : (t + 1) * P],
                                ysT[:, kc, c0 + h : c0 + h + hw],
                                start=(kc == 0), stop=(kc == KC - 1),
                            )
                    nc.scalar.activation(
                        out=est_tiles[t][:, c0 : c0 + w], in_=ps[:, :w],
                        func=AF.Exp, bias=b_shift[:],
                        scale=qs20[:, t : t + 1],
                        accum_out=sparts[:, t, gi : gi + 1],
                    )

            need_tiles = [(c0 + w + P - 1) // P for (c0, w) in groups]

            # ---- y pipeline interleaved with main-loop emission
            NB = 4
            main_emitted = 0
            for j in range(nyt):
                sumsq(nc.vector, ystages[j], kssq[:, j : j + 1])
                if j % NB == NB - 1 or j == nyt - 1:
                    lo = (j // NB) * NB
                    rsqrt_batch(kssq, krs, lo, j + 1, 0.0)
                    # transpose in bursts of up to 4 tiles into one PSUM bank,
                    # then drain with a single wide DVE copy (gpsimd has no
                    # PSUM port, so copies stay on DVE/ScalarE)
                    for b0 in range(lo, j + 1, 4):
                        b1 = min(b0 + 4, j + 1)
                        pt = ptr.tile([P, KC, 4 * P], BF16)
                        for jj in range(b0, b1):
                            sc = scaled.tile([P, D], BF16)
                            nc.vector.tensor_scalar_mul(sc[:], ystages[jj],
                                                         krs[:, jj : jj + 1])
                            o = (jj - b0) * P
                            for kc in range(KC):
                                nc.tensor.transpose(
                                    pt[:, kc, o : o + P],
                                    sc[:, kc * P : (kc + 1) * P], ident[:]
                                )
                        nc.vector.tensor_copy(
                            out=ysT[:, :, b0 * P : b1 * P],
                            in_=pt[:, :, : (b1 - b0) * P])
                    while (main_emitted < ngr
                           and need_tiles[main_emitted] <= j + 1):
                        emit_main_group(main_emitted)
                        main_emitted += 1
            while main_emitted < ngr:
                emit_main_group(main_emitted)
                main_emitted += 1

            # ---- bands (gpsimd fused mult+sum) + S_all reduce (DVE)
            for t in range(T):
                w0 = win_starts[t]
                bscr = band.tile([P, Wtot], F32)
                nc.vector.scalar_tensor_tensor(
                    out=bscr[:], in0=est_tiles[t][:, w0 : w0 + Wtot],
                    scalar=1.0, in1=masks[:, t, :],
                    op0=OP.bypass, op1=OP.mult,
                    accum_out=spos[:, t : t + 1],
                )
                nc.vector.tensor_reduce(out=sall[:, t : t + 1],
                                        in_=sparts[:, t, :],
                                        axis=mybir.AxisListType.X, op=OP.add)

            # ---- batched epilogue: loss rows, mask, partition sum
            sposg = tiny.tile([P, T], F32)
            nc.vector.tensor_add(out=sposg[:], in0=spos[:],
                                 in1=meta_s[:, 2 * T : 3 * T])
            lall = tiny.tile([P, T], F32)
            nc.scalar.activation(out=lall[:], in_=sall[:], func=AF.Ln,
                                 bias=0.0, scale=1.0)
            lpos = tiny.tile([P, T], F32)
            nc.scalar.activation(out=lpos[:], in_=sposg[:], func=AF.Ln,
                                 bias=0.0, scale=1.0)
            dls = tiny.tile([P, T], F32)
            nc.vector.tensor_sub(out=dls[:], in0=lall[:], in1=lpos[:])
            dlm = tiny.tile([P, T], F32)
            nc.vector.tensor_mul(out=dlm[:], in0=dls[:], in1=meta_s[:, T : 2 * T])
            nc.sync.dma_start(out=outp.ap(), in_=dlm[:])

    return nc


def _roundup(a, b):
    return (a + b - 1) // b * b


def plan(valid, ad):
    """Host-side sharding plan from the pad mask / ad ids (index math only)."""
    idx = np.nonzero(valid)[0]
    V = int(idx.size)
    if V == 0:
        return None
    order = idx[np.argsort(ad[idx], kind="stable")]
    ads = ad[order].astype(np.int64)
    R = _roundup(_roundup(V, NCORES) // NCORES, P)
    Vy = _roundup(V, P)
    W = int(np.bincount(ads).max())
    Wtot = min(_roundup(2 * W + P, 32), V)
    T = R // P
    rotate = (R - P + Wtot <= V) and Wtot < V
    if rotate:
        win_starts = tuple(min(t * P, V - Wtot) for t in range(T))
    else:
        Wtot = V
        win_starts = (0,) * T
    return dict(V=V, Vy=Vy, R=R, T=T, W=W, Wtot=Wtot, win_starts=win_starts,
                rotate=rotate, order=order, ads=ads)


def core_inputs(pl, x, y, c):
    """Build core c's input arrays from the plan (host indexing only)."""
    V, R, Vy, W, T, Wtot = (pl["V"], pl["R"], pl["Vy"], pl["W"], pl["T"],
                            pl["Wtot"])
    order, ads, win_starts = pl["order"], pl["ads"], pl["win_starts"]
    g0 = c * R
    take = order[g0 : g0 + R]
    xs = np.zeros((R, D), np.float32)
    xs[: take.size] = x[take]
    nyt = Vy // P
    adr_flat = np.full(R, -1.0, np.float32)
    adr_flat[: take.size] = ads[g0 : g0 + take.size]
    n_valid = max(0, min(R, V - g0))
    rmask_flat = np.zeros(R, np.float32)
    rmask_flat[:n_valid] = 1.0
    # packed [P, T]: column t holds rows [t*P, (t+1)*P) of this core's shard
    adr = np.ascontiguousarray(adr_flat.reshape(T, P).T)
    rmask = np.ascontiguousarray(rmask_flat.reshape(T, P).T)
    rpad = np.ascontiguousarray(1.0 - rmask)
    if pl["rotate"]:
        colsel = (np.arange(V) + g0 - W) % V
    else:
        colsel = np.arange(V)
    cols = order[colsel]
    ys = np.zeros((Vy, D), np.float32)
    ys[:V] = y[cols]
    # partition-major: partition p's tiles contiguous -> 4-5KB DMA descriptors
    xs = np.ascontiguousarray(
        xs.reshape(T, P, D).transpose(1, 0, 2)).reshape(P, T * D)
    ys = np.ascontiguousarray(
        ys.reshape(nyt, P, D).transpose(1, 0, 2)).reshape(P, nyt * D)
    adc = ads[colsel].astype(np.float32)
    MW = 3 * T
    meta = np.zeros((P, MW + T * Wtot), np.float32)
    meta[:, 0:T] = adr
    meta[:, T : 2 * T] = rmask
    meta[:, 2 * T : 3 * T] = rpad
    for t in range(T):
        w0 = win_starts[t]
        meta[:, MW + t * Wtot : MW + (t + 1) * Wtot] = adc[w0 : w0 + Wtot][None, :]
    return {"xs": xs, "ys": ys, "meta": meta}


_prog_cache = {}


def _get_program(key_pl):
    key = (key_pl["V"], key_pl["Vy"], key_pl["R"],
           key_pl["Wtot"], key_pl["win_starts"])
    if key not in _prog_cache:
        _prog_cache[key] = build_program(
            key_pl["V"], key_pl["Vy"], key_pl["R"],
            key_pl["Wtot"], key_pl["win_starts"]
        )
    return _prog_cache[key]


def kernel(logits, labels, pad_mask, ad_idxs, _want_results=False, **run_kwargs):
    x = np.ascontiguousarray(np.asarray(logits), dtype=np.float32).reshape(-1, D)
    y = np.ascontiguousarray(np.asarray(labels), dtype=np.float32).reshape(-1, D)
    valid = np.asarray(pad_mask).reshape(-1).astype(bool)
    ad = np.asarray(ad_idxs).reshape(-1).astype(np.int64)

    pl = plan(valid, ad)
    if pl is None:
        return np.float32(0.0)

    nc = _get_program(pl)
    # CoreSim chokes on the inserted NOPs, so split waits only for the HW path
    if not getattr(nc, "_waits_split", False):
        _split_excess_waits(nc)
        nc._waits_split = True
    in_maps = [core_inputs(pl, x, y, c) for c in range(NCORES)]
    res = run_bass_kernel_spmd(nc, in_maps, core_ids=list(range(NCORES)),
                               **run_kwargs)
    total = sum(float(res.results[c]["partial"].sum()) for c in range(NCORES))
    loss = np.float32(total / pl["V"])
    if _want_results:
        return loss, res
    return loss
